# revision 18
# baseline (speedup 1.0000x reference)
"""WaveNet-style gated dilated conv layer on 8 Trainium2 NeuronCores.

Strategy: data-parallel over batch (B=8 -> 1 batch element per core).
Per core (batch b):
  z_tanh = sum_k Wc_tanh[k] @ x[:, t-d*(2-k)] + Wcond_tanh @ cond + bias
  z_sig  = likewise for the second half of the 2R conv channels
  h      = tanh(z_tanh) * sigmoid(z_sig)
  out    = W_out @ h, skip = W_skip @ h  (1x1 convs)

Precision split (validated against the fp32 reference, gate 2e-2):
 - z_tanh path: bf16 (errors pass through tanh' ~ 1.0)
 - z_sig path: fp8 e4m3 DoubleRow matmuls (errors damped by sigmoid',
   and tanh*sigmoid' <= 0.25) -- rel_err ~1.6e-2 simulated end-to-end.
   Each DoubleRow matmul contracts 256 virtual rows (two 128-row blocks
   paired along the moving operand's free dim) at the SAME ~216ns/512col
   cadence as bf16, so z_sig needs 2 passes instead of 4.
 - out/skip + h: bf16; outputs stored as bf16 (upcast on host).
fp8 sig weights are pre-scaled by 64 on host (keeps small weights out of
the e4m3 subnormal range); the sigmoid activation applies scale=1/64.

DoubleRow moving-operand construction (HW-verified in a microtest):
 - pass 1 contracts (tap0, tap1): one AP over the x8 chunk with the pair
   dimension strided by dilation bytes -- [[part],[4,2],[1,w]] -- i.e.
   pairs (x8[t-8+j], x8[t-4+j]) read from a single buffer, no copies.
 - pass 2 contracts (tap2, cond): x8 and cond8 are loaded as two blocks
   of one [128, 2, pad+gw] tile (single DMA from a host-packed buffer),
   so the natural block stride pairs (x8[t+j], cond8[t+j]).

TRN2 matmul instructions only have room for a single semaphore wait, so
input DMAs are "observed" by the PE via standalone ldweights instructions
before the first matmul that would otherwise combine a DMA wait with a
PSUM WAR wait.

Schedule notes (from NTFF traces):
 - Warmup matmuls bridge from the init barrier (~6.5us) to when the first
   chunks + weights have landed (~12us); ending early lets the PE idle
   >3.4us -> the HAM clock gate re-throttles to 1.2 GHz.
 - Output DMA triggers cost ~0.6us each of serial Sync time and must stay
   on the Sync ring (the Scalar ring stalls the activation FIFO: +12us).
 - The gated multiply runs on GPSIMD (otherwise idle) so the vector
   engine only carries the two PSUM->SBUF output casts.
"""

import sys

for _p in ("/opt/trn_rl_repo",):
    if _p not in sys.path:
        sys.path.append(_p)

from contextlib import ExitStack

import ml_dtypes
import numpy as np

import concourse.bacc as bacc
import concourse.bass as bass
import concourse.tile as tile
from concourse import mybir
from concourse.bass_utils import run_bass_kernel_spmd

B, CIN, T = 8, 128, 16384
R, S, CC, KW = 128, 128, 80, 3
NT = 512           # time-tile width (one PSUM bank of fp32)
N_CORES = 8

BF16 = mybir.dt.bfloat16
FP32 = mybir.dt.float32
F8 = mybir.dt.float8e4
DR = mybir.MatmulPerfMode.DoubleRow
AF = mybir.ActivationFunctionType

WS = 64.0          # fp8 sig-weight prescale (power of 2)
# "fp8_all": z_sig fully fp8 (2 DR passes)      -- fastest, rel ~1.6e-2
# "fp8_01" : z_sig taps0,1 fp8 + tap2/cond bf16 -- safer,   rel ~1.4e-2
# "bf16"   : all-bf16 fallback                  --          rel ~4.9e-3
SIG_MODE = "fp8_all"
MUL_ENGINE = "vector"   # gpsimd tensor_mul measured ~3us/op -- far too slow

_built = {}
_TRACE = False        # set True (e.g. by a test harness) to capture an NTFF profile
_last_results = None  # BassKernelResults of the most recent run


# Streaming chunk widths: small at the head (fast first-compute), large in
# the middle (few DMA triggers), small final chunk so the post-compute
# drain is short.
CHUNK_WIDTHS = [512, 1536] + [2048] * 6 + [1536, 512]
assert sum(CHUNK_WIDTHS) == T
CHUNK_STARTS = [sum(CHUNK_WIDTHS[:i]) for i in range(len(CHUNK_WIDTHS))]
NCH = len(CHUNK_WIDTHS)
PREFETCH = 2         # chunk lookahead beyond the current group
N_WARMUP_MM = 12
# chunks 0-1 compute z_sig in bf16: their fp8 stream would collide with
# the critical bf16+weights loads during the DMA ramp-up and the PE would
# starve -> HAM re-throttle.  From chunk 2 on, DMA has ramped and the fp8
# DoubleRow path carries z_sig.
N_BF16_HEAD_CHUNKS = 2


def _build(dilation: int) -> bass.Bass:
    pad = dilation * (KW - 1)
    sig_fp8 = SIG_MODE in ("fp8_all", "fp8_01")

    nc = bacc.Bacc("TRN2", target_bir_lowering=False, debug=False, num_devices=N_CORES)

    x = nc.declare_dram_parameter("x", [CIN, pad + T], BF16, isOutput=False)
    cond = nc.declare_dram_parameter("cond", [CC, T], BF16, isOutput=False)
    # packed lhsT weights (already transposed to [Cin, Cout] on host)
    wconv = nc.declare_dram_parameter("wconv", [CIN, 2 * KW * R], BF16, isOutput=False)
    wcond = nc.declare_dram_parameter("wcond", [CC, 2 * R], BF16, isOutput=False)
    wos = nc.declare_dram_parameter("wos", [R, R + S], BF16, isOutput=False)
    zbias = nc.declare_dram_parameter("zbias", [R, 2], FP32, isOutput=False)
    SEG = pad + T + 8  # x8 / cond8 region length (+8 slack for the +4-shifted loads)
    if sig_fp8:
        # x8 (causal-padded) ++ cond8 (zero-padded to 128 rows and by pad
        # cols) concatenated on the free dim; per chunk two DMAs fill a
        # [128, 3, pad+gw] tile with blocks (x8, x8+4, cond8+4) so BOTH
        # DoubleRow passes read natural contiguous 3D slices.  (A stride-4
        # overlapping pair AP followed by another DR matmul hangs the PE --
        # HW-bisected; natural block strides are safe.)
        xc8 = nc.declare_dram_parameter("xc8", [CIN, 2 * SEG], F8, isOutput=False)
        wsig1 = nc.declare_dram_parameter("wsig1", [CIN, 2 * R], F8, isOutput=False)
        wsig2 = nc.declare_dram_parameter("wsig2", [CIN, 2 * R], F8, isOutput=False)

    # out and skip concatenated on the free dim: one 3D DMA per chunk
    # flushes both (output triggers cost ~0.6us of serial Sync time each)
    osk = nc.declare_dram_parameter("osk", [R, 2 * T], BF16, isOutput=True)

    with tile.TileContext(nc) as tc, ExitStack() as ctx:
        consts = ctx.enter_context(tc.tile_pool(name="consts", bufs=1))
        inpool = ctx.enter_context(tc.tile_pool(name="inp", bufs=PREFETCH + 2))
        hpool = ctx.enter_context(tc.tile_pool(name="h", bufs=3))
        opool = ctx.enter_context(tc.tile_pool(name="o", bufs=3))
        zpsum = ctx.enter_context(tc.tile_pool(name="zpsum", bufs=2, space="PSUM"))
        opsum = ctx.enter_context(tc.tile_pool(name="opsum", bufs=2, space="PSUM"))

        # Warm-up matmuls kick the PE HAM to 8/8 and two tiny activations
        # trigger the tanh/sigmoid table loads while the first input DMAs
        # fly.  The memsets run on GPSIMD (idle, inits early) so the PE
        # doesn't wait on the busy vector engine.
        garbage = consts.tile([CIN, NT], BF16)
        act_sink = consts.tile([R, 4], FP32)
        nc.gpsimd.memset(garbage[:], 0.0)
        nc.gpsimd.memset(act_sink[:], 0.0)
        for _ in range(N_WARMUP_MM):
            wz = zpsum.tile([R, NT], FP32, tag="ztan")
            nc.tensor.matmul(wz[:], garbage[:, 0:R], garbage[:], start=True, stop=True)
        nc.scalar.activation(act_sink[:], act_sink[:], AF.Tanh)
        nc.scalar.activation(act_sink[:], act_sink[:], AF.Sigmoid)

        # chunk 0 is loaded before anything else: every HWDGE trigger costs
        # ~0.6us of serial sequencer time, so the first-needed data goes first
        xc_tiles = [None] * NCH
        cc_tiles = [None] * NCH
        f8_tiles = [None] * NCH

        if sig_fp8:
            xc8_3d = xc8[:].rearrange("p (a b) -> p a b", a=2)

        def load_bf(g):
            gs, gw = CHUNK_STARTS[g], CHUNK_WIDTHS[g]
            xc = inpool.tile([CIN, pad + gw], BF16, tag="xc")
            nc.sync.dma_start(xc[:], x[:, gs : gs + pad + gw])
            cc = inpool.tile([CC, gw], BF16, tag="cc")
            nc.sync.dma_start(cc[:], cond[:, gs : gs + gw])
            xc_tiles[g], cc_tiles[g] = xc, cc

        def load_f8(g):
            if not sig_fp8 or g < N_BF16_HEAD_CHUNKS:
                return
            gs, gw = CHUNK_STARTS[g], CHUNK_WIDTHS[g]
            f8c = inpool.tile([CIN, 3, pad + gw], F8, tag="f8")
            nc.sync.dma_start(f8c[:, 0:1, :], xc8_3d[:, 0:1, gs : gs + pad + gw])
            nc.sync.dma_start(
                f8c[:, 1:3, :], xc8_3d[:, 0:2, gs + 4 : gs + 4 + pad + gw]
            )
            f8_tiles[g] = f8c

        def load_chunk(g):
            load_bf(g)
            load_f8(g)

        # the z_tanh path's bf16 chunks and the weights are needed first;
        # the fp8 sig stream for chunks 0-1 can land a little later
        load_bf(0)
        load_bf(1)
        wconv_sb = consts.tile([CIN, 2 * KW * R], BF16)
        nc.sync.dma_start(wconv_sb[:], wconv[:])
        wcond_sb = consts.tile([CC, 2 * R], BF16)
        nc.sync.dma_start(wcond_sb[:], wcond[:])
        wos_sb = consts.tile([R, R + S], BF16)
        nc.sync.dma_start(wos_sb[:], wos[:])
        zbias_sb = consts.tile([R, 2], FP32)
        nc.sync.dma_start(zbias_sb[:], zbias[:])
        if sig_fp8:
            wsig1_sb = consts.tile([CIN, 2, R], F8)
            nc.sync.dma_start(wsig1_sb[:, :, :].rearrange("p a b -> p (a b)"), wsig1[:])
            wsig2_sb = consts.tile([CIN, 2, R], F8)
            nc.sync.dma_start(wsig2_sb[:, :, :].rearrange("p a b -> p (a b)"), wsig2[:])


        # Output DMA triggers are deferred by one chunk group so their waits
        # (on the staging copies) are satisfied when the SP sequencer reaches
        # them -- an unsatisfied wait would head-of-line block the input
        # triggers queued behind them on the same HWDGE ring.  (Issuing them
        # from the Scalar ring instead stalls the activation FIFO: +12us.)
        pending_out = []

        osk_3d = osk[:].rearrange("p (a b) -> p a b", a=2)

        def flush_out():
            while pending_out:
                gs, gw, oss = pending_out.pop()
                nc.sync.dma_start(osk_3d[:, 0:2, gs : gs + gw], oss[:, 0:2, :])

        for g in range(NCH):
            gs, gw = CHUNK_STARTS[g], CHUNK_WIDTHS[g]
            for gg in range(g + 1, min(g + PREFETCH + 1, NCH)):
                if xc_tiles[gg] is None:
                    load_chunk(gg)
            xc, cc = xc_tiles[g], cc_tiles[g]
            # let PE observe the chunk DMA sems on standalone ldweights
            # so no accumulating matmul needs two waits
            nc.tensor.ldweights(xc[:, 0:R])
            nc.tensor.ldweights(cc[:, 0:R])
            use_fp8 = sig_fp8 and g >= N_BF16_HEAD_CHUNKS
            if use_fp8:
                f8c = f8_tiles[g]
                f8flat = f8c[:, :, :].rearrange("p a b -> p (a b)")
                nc.tensor.ldweights(f8flat[:, 0:R])
                nc.tensor.ldweights(f8flat[:, (pad + gw) : (pad + gw) + R])

            # out/skip share one staging tile and one 2-bank PSUM tile so a
            # single DVE cast covers both (saves ~260ns/tile of PSUM-read
            # instruction overhead; DVE is the co-bottleneck at ~1.8us/tile)
            oss = opool.tile([R, 2, gw], BF16, tag="oss")
            for ti, l0 in enumerate(range(0, gw, NT)):
                w = min(NT, gw - l0)
                ztan = zpsum.tile([R, w], FP32, tag="ztan")
                zsig = zpsum.tile([R, w], FP32, tag="zsig")
                for k in range(KW):
                    xs = xc[:, l0 + dilation * k : l0 + dilation * k + w]
                    nc.tensor.matmul(
                        ztan[:], wconv_sb[:, k * R : (k + 1) * R], xs,
                        start=(k == 0), stop=False,
                    )
                nc.tensor.matmul(
                    ztan[:], wcond_sb[:, 0:R], cc[:, l0 : l0 + w],
                    start=False, stop=True,
                )
                if use_fp8 and SIG_MODE == "fp8_all":
                    # pass 1: (tap0, tap1) = blocks (x8, x8+4) at offset l0
                    nc.tensor.matmul(
                        zsig[:], wsig1_sb[:, 0:2, :],
                        f8c[:, 0:2, l0 : l0 + w], start=True, stop=False,
                        perf_mode=DR,
                    )
                    # pass 2: (tap2, cond) = blocks (x8+4, cond8+4) at l0+4
                    nc.tensor.matmul(
                        zsig[:], wsig2_sb[:, 0:2, :],
                        f8c[:, 1:3, 4 + l0 : 4 + l0 + w],
                        start=False, stop=True, perf_mode=DR,
                    )
                elif use_fp8 and SIG_MODE == "fp8_01":
                    nc.tensor.matmul(
                        zsig[:], wsig1_sb[:, 0:2, :],
                        f8c[:, 0:2, l0 : l0 + w], start=True, stop=False,
                        perf_mode=DR,
                    )
                    xs = xc[:, l0 + dilation * 2 : l0 + dilation * 2 + w]
                    nc.tensor.matmul(
                        zsig[:], wconv_sb[:, (KW + 2) * R : (KW + 3) * R], xs,
                        start=False, stop=False,
                    )
                    nc.tensor.matmul(
                        zsig[:], wcond_sb[:, R : 2 * R], cc[:, l0 : l0 + w],
                        start=False, stop=True,
                    )
                else:
                    for k in range(KW):
                        xs = xc[:, l0 + dilation * k : l0 + dilation * k + w]
                        nc.tensor.matmul(
                            zsig[:], wconv_sb[:, (KW + k) * R : (KW + k + 1) * R], xs,
                            start=(k == 0), stop=False,
                        )
                    nc.tensor.matmul(
                        zsig[:], wcond_sb[:, R : 2 * R], cc[:, l0 : l0 + w],
                        start=False, stop=True,
                    )

                th = hpool.tile([R, w], BF16, tag="th")
                nc.scalar.activation(th[:], ztan[:], AF.Tanh, bias=zbias_sb[:, 0:1])
                sg = hpool.tile([R, w], BF16, tag="sg")
                sig_scale = (1.0 / WS) if (use_fp8 and SIG_MODE == "fp8_all") else 1.0
                nc.scalar.activation(
                    sg[:], zsig[:], AF.Sigmoid, bias=zbias_sb[:, 1:2], scale=sig_scale
                )
                h = hpool.tile([R, w], BF16, tag="h")
                if MUL_ENGINE == "gpsimd":
                    nc.gpsimd.tensor_mul(h[:], th[:], sg[:])
                else:
                    nc.vector.tensor_mul(h[:], th[:], sg[:])

                for o in range(0, w, NT):
                    ow = min(NT, w - o)
                    pos = opsum.tile([R, 2, ow], FP32, tag="pos")
                    nc.tensor.matmul(
                        pos[:, 0, :], wos_sb[:, 0:R], h[:, o : o + ow],
                        start=True, stop=True,
                    )
                    nc.tensor.matmul(
                        pos[:, 1, :], wos_sb[:, R : R + S], h[:, o : o + ow],
                        start=True, stop=True,
                    )
                    if g == NCH - 1:
                        # the ACT engine is idle at the end; casting there
                        # avoids queueing behind the lagging DVE casts
                        nc.scalar.copy(oss[:, :, l0 + o : l0 + o + ow], pos[:])
                    else:
                        nc.vector.tensor_copy(oss[:, :, l0 + o : l0 + o + ow], pos[:])

            flush_out()
            pending_out.append((gs, gw, oss))
        flush_out()

    nc.compile()
    return nc


def _pack_weights(w_conv, w_cond, w_out, w_skip, b_conv, b_cond):
    bf = ml_dtypes.bfloat16
    wconv_p = np.empty((CIN, 2 * KW * R), dtype=bf)
    for k in range(KW):
        wconv_p[:, k * R : (k + 1) * R] = w_conv[0:R, :, k].T.astype(bf)
        wconv_p[:, (KW + k) * R : (KW + k + 1) * R] = w_conv[R : 2 * R, :, k].T.astype(bf)
    wcond_p = np.concatenate(
        [w_cond[0:R, :, 0].T, w_cond[R : 2 * R, :, 0].T], axis=1
    ).astype(bf)
    wos_p = np.concatenate([w_out[:, :, 0].T, w_skip[:, :, 0].T], axis=1).astype(bf)
    zbias_p = np.stack(
        [b_conv[:R] + b_cond[:R], b_conv[R:] + b_cond[R:]], axis=1
    ).astype(np.float32)
    return wconv_p, wcond_p, wos_p, zbias_p


def _q8(a):
    E4 = ml_dtypes.float8_e4m3
    return np.clip(np.asarray(a, np.float32), -240, 240).astype(E4)


def kernel(**inputs):
    x = np.asarray(inputs["x"], dtype=np.float32)
    cond = np.asarray(inputs["cond"], dtype=np.float32)
    w_conv = np.asarray(inputs["w_conv"], dtype=np.float32)
    b_conv = np.asarray(inputs["b_conv"], dtype=np.float32)
    w_cond = np.asarray(inputs["w_cond"], dtype=np.float32)
    b_cond = np.asarray(inputs["b_cond"], dtype=np.float32)
    w_out = np.asarray(inputs["w_out"], dtype=np.float32)
    b_out = np.asarray(inputs["b_out"], dtype=np.float32)
    w_skip = np.asarray(inputs["w_skip"], dtype=np.float32)
    b_skip = np.asarray(inputs["b_skip"], dtype=np.float32)
    dilation = int(np.asarray(inputs["dilation"]))
    pad = dilation * (KW - 1)
    sig_fp8 = SIG_MODE in ("fp8_all", "fp8_01")

    if dilation not in _built:
        _built[dilation] = _build(dilation)
    nc = _built[dilation]

    wconv_p, wcond_p, wos_p, zbias_p = _pack_weights(
        w_conv, w_cond, w_out, w_skip, b_conv, b_cond
    )
    bf = ml_dtypes.bfloat16
    xb = np.zeros((B, CIN, pad + T), dtype=bf)
    xb[:, :, pad:] = x.astype(bf)
    cb = np.ascontiguousarray(cond.astype(bf))

    in_maps = [
        {
            "x": xb[b],
            "cond": cb[b],
            "wconv": wconv_p,
            "wcond": wcond_p,
            "wos": wos_p,
            "zbias": zbias_p,
        }
        for b in range(B)
    ]
    if sig_fp8:
        E4 = ml_dtypes.float8_e4m3
        # in fp8_01 the DR pass accumulates with unscaled bf16 passes, so
        # its weights must be unscaled too (no post-scale can separate them)
        ws = WS if SIG_MODE == "fp8_all" else 1.0
        SEG = pad + T + 8
        xc8 = np.zeros((B, CIN, 2 * SEG), dtype=E4)
        xc8[:, :, pad : pad + T] = _q8(x)
        xc8[:, :CC, SEG + pad : SEG + pad + T] = _q8(cond)
        # pair weights [p, block, m]: block0/1 interleave along the free dim
        wsig1_p = np.empty((CIN, 2, R), dtype=E4)
        wsig1_p[:, 0, :] = _q8(w_conv[R:, :, 0].T * ws)
        wsig1_p[:, 1, :] = _q8(w_conv[R:, :, 1].T * ws)
        wsig2_p = np.empty((CIN, 2, R), dtype=E4)
        wsig2_p[:, 0, :] = _q8(w_conv[R:, :, 2].T * ws)
        wc_pad = np.zeros((CIN, R), np.float32)
        wc_pad[:CC, :] = w_cond[R:, :, 0].T * ws
        wsig2_p[:, 1, :] = _q8(wc_pad)
        for b in range(B):
            in_maps[b]["xc8"] = xc8[b]
            in_maps[b]["wsig1"] = wsig1_p.reshape(CIN, 2 * R)
            in_maps[b]["wsig2"] = wsig2_p.reshape(CIN, 2 * R)

    br = run_bass_kernel_spmd(nc, in_maps, list(range(N_CORES)), trace=_TRACE)
    global _last_results
    _last_results = br
    res = br.results
    osk = np.stack([res[b]["osk"] for b in range(B)])
    output = osk[:, :, :T].astype(np.float32)
    skip = osk[:, :, T:].astype(np.float32)
    if b_out.any():
        output = output + b_out[None, :, None]
    if b_skip.any():
        skip = skip + b_skip[None, :, None]
    return (output, skip)


# revision 19
# speedup vs baseline: 1.0420x; 1.0420x over previous
"""WaveNet-style gated dilated conv layer on 8 Trainium2 NeuronCores.

Strategy: data-parallel over batch (B=8 -> 1 batch element per core).
Per core (batch b):
  z_tanh = sum_k Wc_tanh[k] @ x[:, t-d*(2-k)] + Wcond_tanh @ cond + bias
  z_sig  = likewise for the second half of the 2R conv channels
  h      = tanh(z_tanh) * sigmoid(z_sig)
  out    = W_out @ h, skip = W_skip @ h  (1x1 convs)

Precision split (validated against the fp32 reference, gate 2e-2):
 - z_tanh path: bf16 (errors pass through tanh' ~ 1.0)
 - z_sig path: fp8 e4m3 DoubleRow matmuls (errors damped by sigmoid',
   and tanh*sigmoid' <= 0.25) -- rel_err ~1.6e-2 simulated end-to-end.
   Each DoubleRow matmul contracts 256 virtual rows (two 128-row blocks
   paired along the moving operand's free dim) at the SAME ~216ns/512col
   cadence as bf16, so z_sig needs 2 passes instead of 4.
 - out/skip + h: bf16; outputs stored as bf16 (upcast on host).
fp8 sig weights are pre-scaled by 64 on host (keeps small weights out of
the e4m3 subnormal range); the sigmoid activation applies scale=1/64.

DoubleRow moving-operand construction (HW-verified in a microtest):
 - pass 1 contracts (tap0, tap1): one AP over the x8 chunk with the pair
   dimension strided by dilation bytes -- [[part],[4,2],[1,w]] -- i.e.
   pairs (x8[t-8+j], x8[t-4+j]) read from a single buffer, no copies.
 - pass 2 contracts (tap2, cond): x8 and cond8 are loaded as two blocks
   of one [128, 2, pad+gw] tile (single DMA from a host-packed buffer),
   so the natural block stride pairs (x8[t+j], cond8[t+j]).

TRN2 matmul instructions only have room for a single semaphore wait, so
input DMAs are "observed" by the PE via standalone ldweights instructions
before the first matmul that would otherwise combine a DMA wait with a
PSUM WAR wait.

Schedule notes (from NTFF traces):
 - Warmup matmuls bridge from the init barrier (~6.5us) to when the first
   chunks + weights have landed (~12us); ending early lets the PE idle
   >3.4us -> the HAM clock gate re-throttles to 1.2 GHz.
 - Output DMA triggers cost ~0.6us each of serial Sync time and must stay
   on the Sync ring (the Scalar ring stalls the activation FIFO: +12us).
 - The gated multiply runs on GPSIMD (otherwise idle) so the vector
   engine only carries the two PSUM->SBUF output casts.
"""

import sys

for _p in ("/opt/trn_rl_repo",):
    if _p not in sys.path:
        sys.path.append(_p)

from contextlib import ExitStack

import ml_dtypes
import numpy as np

import concourse.bacc as bacc
import concourse.bass as bass
import concourse.tile as tile
from concourse import mybir
from concourse.bass_utils import run_bass_kernel_spmd

B, CIN, T = 8, 128, 16384
R, S, CC, KW = 128, 128, 80, 3
NT = 512           # time-tile width (one PSUM bank of fp32)
N_CORES = 8

BF16 = mybir.dt.bfloat16
FP32 = mybir.dt.float32
F8 = mybir.dt.float8e4
DR = mybir.MatmulPerfMode.DoubleRow
AF = mybir.ActivationFunctionType

WS = 64.0          # fp8 sig-weight prescale (power of 2)
# "fp8_all": z_sig fully fp8 (2 DR passes)      -- fastest, rel ~1.6e-2
# "fp8_01" : z_sig taps0,1 fp8 + tap2/cond bf16 -- safer,   rel ~1.4e-2
# "bf16"   : all-bf16 fallback                  --          rel ~4.9e-3
SIG_MODE = "fp8_all"
MUL_ENGINE = "vector"   # gpsimd tensor_mul measured ~3us/op -- far too slow

_built = {}
_TRACE = False        # set True (e.g. by a test harness) to capture an NTFF profile
_last_results = None  # BassKernelResults of the most recent run


# Streaming chunk widths: small at the head (fast first-compute), large in
# the middle (few DMA triggers), small final chunk so the post-compute
# drain is short.
CHUNK_WIDTHS = [512, 1536] + [2048] * 6 + [1536, 512]
assert sum(CHUNK_WIDTHS) == T
CHUNK_STARTS = [sum(CHUNK_WIDTHS[:i]) for i in range(len(CHUNK_WIDTHS))]
NCH = len(CHUNK_WIDTHS)
PREFETCH = 2         # chunk lookahead beyond the current group
N_WARMUP_MM = 20
# chunks 0-1 compute z_sig in bf16: their fp8 stream would collide with
# the critical bf16+weights loads during the DMA ramp-up and the PE would
# starve -> HAM re-throttle.  From chunk 2 on, DMA has ramped and the fp8
# DoubleRow path carries z_sig.
N_BF16_HEAD_CHUNKS = 2


def _build(dilation: int) -> bass.Bass:
    pad = dilation * (KW - 1)
    sig_fp8 = SIG_MODE in ("fp8_all", "fp8_01")

    nc = bacc.Bacc("TRN2", target_bir_lowering=False, debug=False, num_devices=N_CORES)

    x = nc.declare_dram_parameter("x", [CIN, pad + T], BF16, isOutput=False)
    cond = nc.declare_dram_parameter("cond", [CC, T], BF16, isOutput=False)
    # packed lhsT weights (already transposed to [Cin, Cout] on host)
    wconv = nc.declare_dram_parameter("wconv", [CIN, 2 * KW * R], BF16, isOutput=False)
    wcond = nc.declare_dram_parameter("wcond", [CC, 2 * R], BF16, isOutput=False)
    wos = nc.declare_dram_parameter("wos", [R, R + S], BF16, isOutput=False)
    zbias = nc.declare_dram_parameter("zbias", [R, 2], FP32, isOutput=False)
    SEG = pad + T + 8  # x8 / cond8 region length (+8 slack for the +4-shifted loads)
    if sig_fp8:
        # x8 (causal-padded) ++ cond8 (zero-padded to 128 rows and by pad
        # cols) concatenated on the free dim; per chunk two DMAs fill a
        # [128, 3, pad+gw] tile with blocks (x8, x8+4, cond8+4) so BOTH
        # DoubleRow passes read natural contiguous 3D slices.  (A stride-4
        # overlapping pair AP followed by another DR matmul hangs the PE --
        # HW-bisected; natural block strides are safe.)
        xc8 = nc.declare_dram_parameter("xc8", [CIN, 2 * SEG], F8, isOutput=False)
        wsig1 = nc.declare_dram_parameter("wsig1", [CIN, 2 * R], F8, isOutput=False)
        wsig2 = nc.declare_dram_parameter("wsig2", [CIN, 2 * R], F8, isOutput=False)

    out = nc.declare_dram_parameter("out", [R, T], BF16, isOutput=True)
    skip = nc.declare_dram_parameter("skip", [S, T], BF16, isOutput=True)

    with tile.TileContext(nc) as tc, ExitStack() as ctx:
        consts = ctx.enter_context(tc.tile_pool(name="consts", bufs=1))
        inpool = ctx.enter_context(tc.tile_pool(name="inp", bufs=PREFETCH + 2))
        hpool = ctx.enter_context(tc.tile_pool(name="h", bufs=3))
        opool = ctx.enter_context(tc.tile_pool(name="o", bufs=3))
        zpsum = ctx.enter_context(tc.tile_pool(name="zpsum", bufs=2, space="PSUM"))
        opsum = ctx.enter_context(tc.tile_pool(name="opsum", bufs=2, space="PSUM"))

        # Warm-up matmuls kick the PE HAM to 8/8 and two tiny activations
        # trigger the tanh/sigmoid table loads while the first input DMAs
        # fly.  The memsets run on GPSIMD (idle, inits early) so the PE
        # doesn't wait on the busy vector engine.
        garbage = consts.tile([CIN, NT], BF16)
        act_sink = consts.tile([R, 4], FP32)
        nc.gpsimd.memset(garbage[:], 0.0)
        nc.gpsimd.memset(act_sink[:], 0.0)
        for _ in range(N_WARMUP_MM):
            wz = zpsum.tile([R, NT], FP32, tag="ztan")
            nc.tensor.matmul(wz[:], garbage[:, 0:R], garbage[:], start=True, stop=True)
        nc.scalar.activation(act_sink[:], act_sink[:], AF.Tanh)
        nc.scalar.activation(act_sink[:], act_sink[:], AF.Sigmoid)

        # chunk 0 is loaded before anything else: every HWDGE trigger costs
        # ~0.6us of serial sequencer time, so the first-needed data goes first
        xc_tiles = [None] * NCH
        cc_tiles = [None] * NCH
        f8_tiles = [None] * NCH

        if sig_fp8:
            xc8_3d = xc8[:].rearrange("p (a b) -> p a b", a=2)

        def load_bf(g):
            gs, gw = CHUNK_STARTS[g], CHUNK_WIDTHS[g]
            xc = inpool.tile([CIN, pad + gw], BF16, tag="xc")
            nc.sync.dma_start(xc[:], x[:, gs : gs + pad + gw])
            cc = inpool.tile([CC, gw], BF16, tag="cc")
            nc.sync.dma_start(cc[:], cond[:, gs : gs + gw])
            xc_tiles[g], cc_tiles[g] = xc, cc

        def load_f8(g):
            if not sig_fp8 or g < N_BF16_HEAD_CHUNKS:
                return
            gs, gw = CHUNK_STARTS[g], CHUNK_WIDTHS[g]
            f8c = inpool.tile([CIN, 3, pad + gw], F8, tag="f8")
            nc.sync.dma_start(f8c[:, 0:1, :], xc8_3d[:, 0:1, gs : gs + pad + gw])
            nc.sync.dma_start(
                f8c[:, 1:3, :], xc8_3d[:, 0:2, gs + 4 : gs + 4 + pad + gw]
            )
            f8_tiles[g] = f8c

        def load_chunk(g):
            load_bf(g)
            load_f8(g)

        # the z_tanh path's bf16 chunks and the weights are needed first;
        # the fp8 sig stream for chunks 0-1 can land a little later
        load_bf(0)
        load_bf(1)
        wconv_sb = consts.tile([CIN, 2 * KW * R], BF16)
        nc.sync.dma_start(wconv_sb[:], wconv[:])
        wcond_sb = consts.tile([CC, 2 * R], BF16)
        nc.sync.dma_start(wcond_sb[:], wcond[:])
        wos_sb = consts.tile([R, R + S], BF16)
        nc.sync.dma_start(wos_sb[:], wos[:])
        zbias_sb = consts.tile([R, 2], FP32)
        nc.sync.dma_start(zbias_sb[:], zbias[:])
        if sig_fp8:
            wsig1_sb = consts.tile([CIN, 2, R], F8)
            nc.sync.dma_start(wsig1_sb[:, :, :].rearrange("p a b -> p (a b)"), wsig1[:])
            wsig2_sb = consts.tile([CIN, 2, R], F8)
            nc.sync.dma_start(wsig2_sb[:, :, :].rearrange("p a b -> p (a b)"), wsig2[:])


        # Output DMA triggers are deferred by one chunk group so their waits
        # (on the staging copies) are satisfied when the SP sequencer reaches
        # them -- an unsatisfied wait would head-of-line block the input
        # triggers queued behind them on the same HWDGE ring.  (Issuing them
        # from the Scalar ring instead stalls the activation FIFO: +12us.)
        pending_out = []

        def flush_out():
            while pending_out:
                gs, gw, oss = pending_out.pop()
                nc.sync.dma_start(out[:, gs : gs + gw], oss[:, 0, :])
                nc.sync.dma_start(skip[:, gs : gs + gw], oss[:, 1, :])

        for g in range(NCH):
            gs, gw = CHUNK_STARTS[g], CHUNK_WIDTHS[g]
            for gg in range(g + 1, min(g + PREFETCH + 1, NCH)):
                if xc_tiles[gg] is None:
                    load_chunk(gg)
            xc, cc = xc_tiles[g], cc_tiles[g]
            # let PE observe the chunk DMA sems on standalone ldweights
            # so no accumulating matmul needs two waits
            nc.tensor.ldweights(xc[:, 0:R])
            nc.tensor.ldweights(cc[:, 0:R])
            use_fp8 = sig_fp8 and g >= N_BF16_HEAD_CHUNKS
            if use_fp8:
                f8c = f8_tiles[g]
                f8flat = f8c[:, :, :].rearrange("p a b -> p (a b)")
                nc.tensor.ldweights(f8flat[:, 0:R])
                nc.tensor.ldweights(f8flat[:, (pad + gw) : (pad + gw) + R])

            # out/skip share one staging tile and one 2-bank PSUM tile so a
            # single DVE cast covers both (saves ~260ns/tile of PSUM-read
            # instruction overhead; DVE is the co-bottleneck at ~1.8us/tile)
            oss = opool.tile([R, 2, gw], BF16, tag="oss")
            for ti, l0 in enumerate(range(0, gw, NT)):
                w = min(NT, gw - l0)
                ztan = zpsum.tile([R, w], FP32, tag="ztan")
                zsig = zpsum.tile([R, w], FP32, tag="zsig")
                for k in range(KW):
                    xs = xc[:, l0 + dilation * k : l0 + dilation * k + w]
                    nc.tensor.matmul(
                        ztan[:], wconv_sb[:, k * R : (k + 1) * R], xs,
                        start=(k == 0), stop=False,
                    )
                nc.tensor.matmul(
                    ztan[:], wcond_sb[:, 0:R], cc[:, l0 : l0 + w],
                    start=False, stop=True,
                )
                if use_fp8 and SIG_MODE == "fp8_all":
                    # pass 1: (tap0, tap1) = blocks (x8, x8+4) at offset l0
                    nc.tensor.matmul(
                        zsig[:], wsig1_sb[:, 0:2, :],
                        f8c[:, 0:2, l0 : l0 + w], start=True, stop=False,
                        perf_mode=DR,
                    )
                    # pass 2: (tap2, cond) = blocks (x8+4, cond8+4) at l0+4
                    nc.tensor.matmul(
                        zsig[:], wsig2_sb[:, 0:2, :],
                        f8c[:, 1:3, 4 + l0 : 4 + l0 + w],
                        start=False, stop=True, perf_mode=DR,
                    )
                elif use_fp8 and SIG_MODE == "fp8_01":
                    nc.tensor.matmul(
                        zsig[:], wsig1_sb[:, 0:2, :],
                        f8c[:, 0:2, l0 : l0 + w], start=True, stop=False,
                        perf_mode=DR,
                    )
                    xs = xc[:, l0 + dilation * 2 : l0 + dilation * 2 + w]
                    nc.tensor.matmul(
                        zsig[:], wconv_sb[:, (KW + 2) * R : (KW + 3) * R], xs,
                        start=False, stop=False,
                    )
                    nc.tensor.matmul(
                        zsig[:], wcond_sb[:, R : 2 * R], cc[:, l0 : l0 + w],
                        start=False, stop=True,
                    )
                else:
                    for k in range(KW):
                        xs = xc[:, l0 + dilation * k : l0 + dilation * k + w]
                        nc.tensor.matmul(
                            zsig[:], wconv_sb[:, (KW + k) * R : (KW + k + 1) * R], xs,
                            start=(k == 0), stop=False,
                        )
                    nc.tensor.matmul(
                        zsig[:], wcond_sb[:, R : 2 * R], cc[:, l0 : l0 + w],
                        start=False, stop=True,
                    )

                th = hpool.tile([R, w], BF16, tag="th")
                nc.scalar.activation(th[:], ztan[:], AF.Tanh, bias=zbias_sb[:, 0:1])
                sg = hpool.tile([R, w], BF16, tag="sg")
                sig_scale = (1.0 / WS) if (use_fp8 and SIG_MODE == "fp8_all") else 1.0
                nc.scalar.activation(
                    sg[:], zsig[:], AF.Sigmoid, bias=zbias_sb[:, 1:2], scale=sig_scale
                )
                h = hpool.tile([R, w], BF16, tag="h")
                if MUL_ENGINE == "gpsimd":
                    nc.gpsimd.tensor_mul(h[:], th[:], sg[:])
                else:
                    nc.vector.tensor_mul(h[:], th[:], sg[:])

                for o in range(0, w, NT):
                    ow = min(NT, w - o)
                    pos = opsum.tile([R, 2, ow], FP32, tag="pos")
                    nc.tensor.matmul(
                        pos[:, 0, :], wos_sb[:, 0:R], h[:, o : o + ow],
                        start=True, stop=True,
                    )
                    nc.tensor.matmul(
                        pos[:, 1, :], wos_sb[:, R : R + S], h[:, o : o + ow],
                        start=True, stop=True,
                    )
                    if g == NCH - 1:
                        # the ACT engine is idle at the end; casting there
                        # avoids queueing behind the lagging DVE casts
                        nc.scalar.copy(oss[:, :, l0 + o : l0 + o + ow], pos[:])
                    else:
                        nc.vector.tensor_copy(oss[:, :, l0 + o : l0 + o + ow], pos[:])

            flush_out()
            pending_out.append((gs, gw, oss))
        flush_out()

    nc.compile()
    return nc


def _pack_weights(w_conv, w_cond, w_out, w_skip, b_conv, b_cond):
    bf = ml_dtypes.bfloat16
    wconv_p = np.empty((CIN, 2 * KW * R), dtype=bf)
    for k in range(KW):
        wconv_p[:, k * R : (k + 1) * R] = w_conv[0:R, :, k].T.astype(bf)
        wconv_p[:, (KW + k) * R : (KW + k + 1) * R] = w_conv[R : 2 * R, :, k].T.astype(bf)
    wcond_p = np.concatenate(
        [w_cond[0:R, :, 0].T, w_cond[R : 2 * R, :, 0].T], axis=1
    ).astype(bf)
    wos_p = np.concatenate([w_out[:, :, 0].T, w_skip[:, :, 0].T], axis=1).astype(bf)
    zbias_p = np.stack(
        [b_conv[:R] + b_cond[:R], b_conv[R:] + b_cond[R:]], axis=1
    ).astype(np.float32)
    return wconv_p, wcond_p, wos_p, zbias_p


def _q8(a):
    E4 = ml_dtypes.float8_e4m3
    return np.clip(np.asarray(a, np.float32), -240, 240).astype(E4)


def kernel(**inputs):
    x = np.asarray(inputs["x"], dtype=np.float32)
    cond = np.asarray(inputs["cond"], dtype=np.float32)
    w_conv = np.asarray(inputs["w_conv"], dtype=np.float32)
    b_conv = np.asarray(inputs["b_conv"], dtype=np.float32)
    w_cond = np.asarray(inputs["w_cond"], dtype=np.float32)
    b_cond = np.asarray(inputs["b_cond"], dtype=np.float32)
    w_out = np.asarray(inputs["w_out"], dtype=np.float32)
    b_out = np.asarray(inputs["b_out"], dtype=np.float32)
    w_skip = np.asarray(inputs["w_skip"], dtype=np.float32)
    b_skip = np.asarray(inputs["b_skip"], dtype=np.float32)
    dilation = int(np.asarray(inputs["dilation"]))
    pad = dilation * (KW - 1)
    sig_fp8 = SIG_MODE in ("fp8_all", "fp8_01")

    if dilation not in _built:
        _built[dilation] = _build(dilation)
    nc = _built[dilation]

    wconv_p, wcond_p, wos_p, zbias_p = _pack_weights(
        w_conv, w_cond, w_out, w_skip, b_conv, b_cond
    )
    bf = ml_dtypes.bfloat16
    xb = np.zeros((B, CIN, pad + T), dtype=bf)
    xb[:, :, pad:] = x.astype(bf)
    cb = np.ascontiguousarray(cond.astype(bf))

    in_maps = [
        {
            "x": xb[b],
            "cond": cb[b],
            "wconv": wconv_p,
            "wcond": wcond_p,
            "wos": wos_p,
            "zbias": zbias_p,
        }
        for b in range(B)
    ]
    if sig_fp8:
        E4 = ml_dtypes.float8_e4m3
        # in fp8_01 the DR pass accumulates with unscaled bf16 passes, so
        # its weights must be unscaled too (no post-scale can separate them)
        ws = WS if SIG_MODE == "fp8_all" else 1.0
        SEG = pad + T + 8
        xc8 = np.zeros((B, CIN, 2 * SEG), dtype=E4)
        xc8[:, :, pad : pad + T] = _q8(x)
        xc8[:, :CC, SEG + pad : SEG + pad + T] = _q8(cond)
        # pair weights [p, block, m]: block0/1 interleave along the free dim
        wsig1_p = np.empty((CIN, 2, R), dtype=E4)
        wsig1_p[:, 0, :] = _q8(w_conv[R:, :, 0].T * ws)
        wsig1_p[:, 1, :] = _q8(w_conv[R:, :, 1].T * ws)
        wsig2_p = np.empty((CIN, 2, R), dtype=E4)
        wsig2_p[:, 0, :] = _q8(w_conv[R:, :, 2].T * ws)
        wc_pad = np.zeros((CIN, R), np.float32)
        wc_pad[:CC, :] = w_cond[R:, :, 0].T * ws
        wsig2_p[:, 1, :] = _q8(wc_pad)
        for b in range(B):
            in_maps[b]["xc8"] = xc8[b]
            in_maps[b]["wsig1"] = wsig1_p.reshape(CIN, 2 * R)
            in_maps[b]["wsig2"] = wsig2_p.reshape(CIN, 2 * R)

    br = run_bass_kernel_spmd(nc, in_maps, list(range(N_CORES)), trace=_TRACE)
    global _last_results
    _last_results = br
    res = br.results
    output = np.stack([res[b]["out"] for b in range(B)]).astype(np.float32)
    skip = np.stack([res[b]["skip"] for b in range(B)]).astype(np.float32)
    if b_out.any():
        output = output + b_out[None, :, None]
    if b_skip.any():
        skip = skip + b_skip[None, :, None]
    return (output, skip)
